# revision 3
# baseline (speedup 1.0000x reference)
"""Trainium2 Bass kernel for nn_KsModel_44049184588156 (2-layer GAT + projection head).

Sharding: pure data-parallel over the batch axis — one batch element per
NeuronCore (bs=8 over 8 cores), weights replicated, no collectives.

Per-core compute is laid out transposed ([feature/j on the 128 partitions,
node index i on the free axis]) so that
  - the softmax normalization axis (the *first* node index of the [N, N]
    attention logits) becomes a free-axis reduction,
  - every matmul's stationary operand (lhsT) is already in [K, M] layout,
  - the adjacency mask is transposed once (PE transposes) and reused by all
    6 heads and the out-attention layer.

Per attention instance (6 heads + 1 out-att), per j-chunk (128 rows of the
transposed [N, N] logit matrix):
  e^T[j,i] = f[i] + g[j] + BIGNEG*(1-adj^T[j,i])  -- 3 accumulating matmuls
             (two K=1 broadcast outer products + identity-move of the mask)
  exp(leaky(e)) = max(exp(e), exp(0.2*e))          -- two ScalarE passes
  M^T, colsum = scalar_tensor_tensor(max, accum)   -- one VectorE pass
  hp^T += (Wh[j,:]/colsum[j])^T-matmul M^T         -- PSUM accumulation
The softmax denominator (a sum over the first index = column sum) is folded
into Wh's rows, so the [N, N] matrix is never normalized elementwise.
"""

import numpy as np

import concourse.bass as bass
import concourse.mybir as mybir
import concourse.tile as tile
from concourse import bacc
from concourse.bass_utils import run_bass_kernel_spmd
from concourse.masks import make_identity

dt = mybir.dt
F32 = dt.float32
F32R = dt.float32r
BF16 = dt.bfloat16
I32 = dt.int32
Alu = mybir.AluOpType
Act = mybir.ActivationFunctionType

BS, N, F_IN, D, H = 8, 1024, 768, 128, 6
HID = D * H  # 768
P = 128
NI = N // P       # 8 i-chunks
NJ = N // P       # 8 j-chunks
KF = F_IN // P    # 6 feature chunks
KO = HID // P     # 6 hidden chunks
BIGNEG = -1.0e5   # additive mask; 0.2*BIGNEG << -88 so exp() flushes to 0
ALPHA = 0.2

TRACE = False  # set by the test harness for profiling


def r(ap):
    return ap.bitcast(F32R)


def _attention_tiles(nc, tc, ep_ps, work, f_sb, g_sb, adjT, ones_r, ident_bf,
                     moT_dst, rc_dst, jc, tag):
    """One j-chunk of masked exp(leaky(f_i + g_j)) in transposed layout.

    Writes the [P, N] tile into moT_dst and 1/colsum into rc_dst ([P, 1]).
    """
    ep = ep_ps.tile([P, N], F32, tag=f"ep{tag}")
    for s in range(2):
        sl = slice(s * 512, (s + 1) * 512)
        nc.tensor.matmul(ep[:, sl], ones_r[0:1, 0:P], f_sb[0:1, sl],
                         start=True, stop=False)
        nc.tensor.matmul(ep[:, sl], g_sb[0:1, jc * P:(jc + 1) * P],
                         ones_r[0:1, sl], start=False, stop=False)
        nc.tensor.matmul(ep[:, sl], ident_bf, adjT[:, jc, sl],
                         start=False, stop=True)
    e1 = work.tile([P, N], F32, tag=f"e1{tag}", bufs=2)
    e2 = work.tile([P, N], F32, tag=f"e2{tag}", bufs=2)
    nc.scalar.activation(out=e1, in_=ep, func=Act.Exp)
    nc.scalar.activation(out=e2, in_=ep, func=Act.Exp, scale=ALPHA)
    cs = work.tile([P, 1], F32, tag=f"cs{tag}", bufs=2)
    nc.vector.scalar_tensor_tensor(out=moT_dst, in0=e1, scalar=0.0, in1=e2,
                                   op0=Alu.bypass, op1=Alu.max, accum_out=cs)
    csm = work.tile([P, 1], F32, tag=f"csm{tag}", bufs=2)
    nc.vector.tensor_scalar(out=csm, in0=cs, scalar1=1e-30, scalar2=None,
                            op0=Alu.max)
    nc.vector.reciprocal(out=rc_dst, in_=csm)


def _build_nc():
    nc = bacc.Bacc("TRN2", target_bir_lowering=False, debug=False)

    x_d = nc.dram_tensor("x", [N, F_IN], F32, kind="ExternalInput").ap()
    adj_d = nc.dram_tensor("adj", [N, N], I32, kind="ExternalInput").ap()
    nm_d = nc.dram_tensor("nm", [1, N], I32, kind="ExternalInput").ap()
    w_d = nc.dram_tensor("W", [H, F_IN, D], F32, kind="ExternalInput").ap()
    a1_d = nc.dram_tensor("a1", [H, D], F32, kind="ExternalInput").ap()
    a2_d = nc.dram_tensor("a2", [H, D], F32, kind="ExternalInput").ap()
    wo_d = nc.dram_tensor("Wo", [HID, HID], F32, kind="ExternalInput").ap()
    ao1_d = nc.dram_tensor("ao1", [1, HID], F32, kind="ExternalInput").ap()
    ao2_d = nc.dram_tensor("ao2", [1, HID], F32, kind="ExternalInput").ap()
    wp_d = nc.dram_tensor("Wp", [F_IN, HID], F32, kind="ExternalInput").ap()
    bp_d = nc.dram_tensor("bp", [1, HID], F32, kind="ExternalInput").ap()
    kw_d = nc.dram_tensor("kw", [HID, 1], F32, kind="ExternalInput").ap()
    kb_d = nc.dram_tensor("kb", [1, 1], F32, kind="ExternalInput").ap()
    out_d = nc.dram_tensor("out", [1, N], F32, kind="ExternalOutput").ap()

    with tile.TileContext(nc) as tc:
        with tc.tile_pool(name="pers", bufs=1) as pers:
            # ---- tiny persistent constants ----
            ident = pers.tile([P, P], F32, tag="ident")
            make_identity(nc, ident)
            ident_r = pers.tile([P, P], F32R, tag="ident_r")
            nc.vector.tensor_copy(ident_r, ident)
            ident_bf = pers.tile([P, P], BF16, tag="ident_bf")
            nc.vector.tensor_copy(ident_bf, ident)
            ones_r = pers.tile([1, N], F32R, tag="ones_r")
            nm_sb = pers.tile([1, N], I32, tag="nm")
            nc.sync.dma_start(out=nm_sb, in_=nm_d)
            kb_sb = pers.tile([1, 1], F32, tag="kb")
            nc.sync.dma_start(out=kb_sb, in_=kb_d)
            kb_r = pers.tile([1, 1], F32R, tag="kb_r")
            nc.vector.tensor_copy(kb_r, kb_sb)
            # [p, h, 2]: col 0 = a1[h], col 1 = a2[h]
            al_sb = pers.tile([P, H, 2], F32R, tag="al")
            nc.sync.dma_start(out=al_sb[:, :, 0:1],
                              in_=r(a1_d).rearrange("h d -> d h").unsqueeze(2))
            nc.sync.dma_start(out=al_sb[:, :, 1:2],
                              in_=r(a2_d).rearrange("h d -> d h").unsqueeze(2))
            ao_sb = pers.tile([P, KO, 2], F32R, tag="ao")
            nc.sync.dma_start(out=ao_sb[:, :, 0:1],
                              in_=r(ao1_d).rearrange("o (c p) -> p c o", p=P))
            nc.sync.dma_start(out=ao_sb[:, :, 1:2],
                              in_=r(ao2_d).rearrange("o (c p) -> p c o", p=P))
            bp_sb = pers.tile([P, KO], F32, tag="bp")
            nc.sync.dma_start(out=bp_sb,
                              in_=bp_d.rearrange("o (c p) -> p (c o)", p=P))
            kw_sb = pers.tile([P, KO], F32R, tag="kw")
            nc.sync.dma_start(out=kw_sb,
                              in_=r(kw_d).rearrange("(c p) o -> p (c o)", p=P))

            whT = pers.tile([P, H, N], F32R, tag="whT")      # Wh^T per head [d, i]
            projT = pers.tile([P, KO, N], F32, tag="projT")  # proj^T [o, i]

            # ============ Phase A: x^T, Wh^T, proj^T ============
            with tc.tile_pool(name="pha", bufs=1) as pha, \
                 tc.tile_pool(name="pha_tr", bufs=4, space="PSUM") as pha_tr, \
                 tc.tile_pool(name="pha_mm", bufs=4, space="PSUM") as pha_mm:
                ones_f = pha.tile([1, N], F32, tag="ones_f")
                nc.vector.memset(ones_f, 1.0)
                nc.vector.tensor_copy(ones_r, ones_f)

                x_sb = pha.tile([P, NI, F_IN], F32, tag="x")
                nc.sync.dma_start(out=x_sb,
                                  in_=x_d.rearrange("(c p) f -> p c f", p=P))
                w_sb = pha.tile([P, H, KF, D], F32R, tag="w")
                nc.sync.dma_start(out=w_sb,
                                  in_=r(w_d).rearrange("h (c p) d -> p h c d", p=P))
                wp_sb = pha.tile([P, KF, HID], F32R, tag="wp")
                nc.sync.dma_start(out=wp_sb,
                                  in_=r(wp_d).rearrange("(c p) o -> p c o", p=P))

                xT = pha.tile([P, KF, N], F32R, tag="xT")
                xTre = pha.tile([P, KF, N], F32R, tag="xTre")  # relu(x)^T
                for kf in range(KF):
                    for icg in range(NI // 4):
                        tp = pha_tr.tile([P, 512], F32, tag="xtr")
                        for q in range(4):
                            ic = icg * 4 + q
                            nc.tensor.transpose(
                                tp[:, q * P:(q + 1) * P],
                                x_sb[:, ic, kf * P:(kf + 1) * P], ident)
                        sl = slice(icg * 512, (icg + 1) * 512)
                        nc.any.tensor_copy(xT[:, kf, sl], tp)
                        nc.vector.tensor_scalar(out=xTre[:, kf, sl], in0=tp,
                                                scalar1=0.0, scalar2=None,
                                                op0=Alu.max)

                for h in range(H):
                    for s in range(2):
                        sl = slice(s * 512, (s + 1) * 512)
                        mp = pha_mm.tile([P, 512], F32, tag="mm512")
                        for kf in range(KF):
                            nc.tensor.matmul(mp, w_sb[:, h, kf, :], xT[:, kf, sl],
                                             start=(kf == 0), stop=(kf == KF - 1))
                        nc.any.tensor_copy(whT[:, h, sl], mp)

                for oc in range(KO):
                    for s in range(2):
                        sl = slice(s * 512, (s + 1) * 512)
                        mp = pha_mm.tile([P, 512], F32, tag="mm512")
                        for kf in range(KF):
                            nc.tensor.matmul(mp, wp_sb[:, kf, oc * P:(oc + 1) * P],
                                             xTre[:, kf, sl],
                                             start=(kf == 0), stop=(kf == KF - 1))
                        nc.scalar.activation(out=projT[:, oc, sl], in_=mp,
                                             func=Act.Identity,
                                             bias=bp_sb[:, oc:oc + 1])

            # ============ Phase B: transposed additive adjacency mask ============
            with tc.tile_pool(name="adjTp", bufs=1) as adjTp:
                adjT = adjTp.tile([P, NJ, N], BF16, tag="adjT")
                with tc.tile_pool(name="phb", bufs=1) as phb, \
                     tc.tile_pool(name="phb_ps", bufs=1, space="PSUM") as phb_ps:
                    adj_sb = phb.tile([P, NI, N], I32, tag="adj")
                    nc.sync.dma_start(out=adj_sb,
                                      in_=adj_d.rearrange("(c p) j -> p c j", p=P))
                    adjf = phb.tile([P, NI, N], BF16, tag="adjf")
                    for ic in range(NI):
                        # BIGNEG * (1 - adj) = adj*1e5 - 1e5
                        nc.vector.tensor_scalar(out=adjf[:, ic, :],
                                                in0=adj_sb[:, ic, :],
                                                scalar1=-BIGNEG, scalar2=BIGNEG,
                                                op0=Alu.mult, op1=Alu.add)
                    for icg in range(NI // 4):
                        tps = []
                        for jc in range(NJ):
                            tp = phb_ps.tile([P, 512], BF16, tag=f"atr{jc}")
                            for q in range(4):
                                ic = icg * 4 + q
                                nc.tensor.transpose(
                                    tp[:, q * P:(q + 1) * P],
                                    adjf[:, ic, jc * P:(jc + 1) * P], ident_bf)
                            tps.append(tp)
                        sl = slice(icg * 512, (icg + 1) * 512)
                        for jc in range(NJ):
                            nc.any.tensor_copy(adjT[:, jc, sl], tps[jc])

                # ---- long-lived layer-2 tensors ----
                with tc.tile_pool(name="big2", bufs=1) as big2:
                    whoT = big2.tile([P, KO, N], F32R, tag="whoT")
                    moT = big2.tile([P, NJ, N], BF16, tag="moT")
                    rc2 = big2.tile([P, NJ], F32, tag="rc2")
                    fo_sb = big2.tile([1, N], F32R, tag="fo")
                    go_sb = big2.tile([1, N], F32R, tag="go")

                    # ============ Phase C: layer-1 heads ============
                    with tc.tile_pool(name="x2Tp", bufs=1) as x2Tp:
                        x2T = x2Tp.tile([P, H, N], F32R, tag="x2T")
                        for h in range(H):
                            _layer1_head(nc, tc, h, whT, adjT, al_sb, ones_r,
                                         ident_r, ident_bf, x2T)

                        # ============ Phase D1: Who^T ============
                        with tc.tile_pool(name="phd1", bufs=1) as phd1, \
                             tc.tile_pool(name="phd1_ps", bufs=4,
                                          space="PSUM") as d1_ps:
                            wo_sb = phd1.tile([P, KO, HID], F32R, tag="wo")
                            nc.sync.dma_start(
                                out=wo_sb,
                                in_=r(wo_d).rearrange("(c p) o -> p c o", p=P))
                            for oc in range(KO):
                                for s in range(2):
                                    sl = slice(s * 512, (s + 1) * 512)
                                    mp = d1_ps.tile([P, 512], F32, tag="womm")
                                    for kc in range(KO):
                                        nc.tensor.matmul(
                                            mp, wo_sb[:, kc, oc * P:(oc + 1) * P],
                                            x2T[:, kc, sl],
                                            start=(kc == 0), stop=(kc == KO - 1))
                                    nc.any.tensor_copy(whoT[:, oc, sl], mp)

                    # ============ Phase D2: fo/go + masked exp tiles ============
                    with tc.tile_pool(name="phd2", bufs=1) as phd2, \
                         tc.tile_pool(name="phd2_fg", bufs=2, space="PSUM") as d2_fg, \
                         tc.tile_pool(name="phd2_ps", bufs=2, space="PSUM") as d2_ps:
                        for col, dest in ((0, fo_sb), (1, go_sb)):
                            for s in range(2):
                                sl = slice(s * 512, (s + 1) * 512)
                                fp = d2_fg.tile([1, 512], F32, tag="fgo")
                                for oc in range(KO):
                                    nc.tensor.matmul(
                                        fp, ao_sb[:, oc, col:col + 1],
                                        whoT[:, oc, sl],
                                        start=(oc == 0), stop=(oc == KO - 1))
                                nc.any.tensor_copy(dest[0:1, sl], fp)
                        for jc in range(NJ):
                            _attention_tiles(nc, tc, d2_ps, phd2, fo_sb, go_sb,
                                             adjT, ones_r, ident_bf,
                                             moT[:, jc, :], rc2[:, jc:jc + 1],
                                             jc, "b")

                    # ============ Phase D3: ho^T, elu, log_softmax, know ============
                    with tc.tile_pool(name="phd3", bufs=1) as phd3, \
                         tc.tile_pool(name="phd3_w", bufs=2) as phd3w, \
                         tc.tile_pool(name="phd3_tr", bufs=2, space="PSUM") as d3_tr, \
                         tc.tile_pool(name="phd3_ho", bufs=2, space="PSUM") as ho_ps, \
                         tc.tile_pool(name="phd3_kp", bufs=1, space="PSUM") as kp_ps:
                        kp = kp_ps.tile([1, N], F32, tag="kp")
                        for oc in range(KO):
                            ho = ho_ps.tile([P, N], F32, tag="ho")
                            for jc in range(NJ):
                                # Who[j, oc-block] via on-demand transpose
                                tp = d3_tr.tile([P, P], F32R, tag="whotr")
                                nc.tensor.transpose(
                                    tp, whoT[:, oc, jc * P:(jc + 1) * P], ident_r)
                                whop = phd3w.tile([P, D], BF16, tag="whop")
                                nc.vector.tensor_scalar(
                                    out=whop, in0=tp, scalar1=rc2[:, jc:jc + 1],
                                    scalar2=None, op0=Alu.mult)
                                for s in range(2):
                                    sl = slice(s * 512, (s + 1) * 512)
                                    nc.tensor.matmul(ho[:, sl], whop,
                                                     moT[:, jc, sl],
                                                     start=(jc == 0),
                                                     stop=(jc == NJ - 1))
                            # elu
                            es = phd3.tile([P, N], F32, tag="es2")
                            nc.scalar.activation(out=es, in_=ho, func=Act.Exp)
                            tm = phd3.tile([P, N], F32, tag="tm2")
                            nc.vector.tensor_scalar(out=tm, in0=es, scalar1=-1.0,
                                                    scalar2=0.0, op0=Alu.add,
                                                    op1=Alu.min)
                            eho = phd3.tile([P, N], F32, tag="eho", bufs=2)
                            nc.vector.scalar_tensor_tensor(out=eho, in0=ho,
                                                           scalar=0.0, in1=tm,
                                                           op0=Alu.max, op1=Alu.add)
                            # log_softmax over i (free axis), then + projT
                            nmx = phd3.tile([P, 1], F32, tag="nmx", bufs=2)
                            nc.vector.tensor_reduce(out=nmx, in_=eho,
                                                    axis=mybir.AxisListType.X,
                                                    op=Alu.max, negate=True)
                            scr = phd3.tile([P, N], F32, tag="scr")
                            ssum = phd3.tile([P, 1], F32, tag="ssum", bufs=2)
                            nc.scalar.activation(out=scr, in_=eho, func=Act.Exp,
                                                 bias=nmx, accum_out=ssum)
                            lsum = phd3.tile([P, 1], F32, tag="lsum", bufs=2)
                            nc.scalar.activation(out=lsum, in_=ssum, func=Act.Ln)
                            shift = phd3.tile([P, 1], F32, tag="shift", bufs=2)
                            nc.vector.tensor_sub(shift, nmx, lsum)
                            hid = phd3.tile([P, N], F32R, tag="hid", bufs=2)
                            nc.vector.scalar_tensor_tensor(
                                out=hid, in0=eho, scalar=shift,
                                in1=projT[:, oc, :], op0=Alu.add, op1=Alu.add)
                            for s in range(2):
                                sl = slice(s * 512, (s + 1) * 512)
                                nc.tensor.matmul(kp[0:1, sl], kw_sb[:, oc:oc + 1],
                                                 hid[:, sl],
                                                 start=(oc == 0), stop=False)
                        for s in range(2):
                            sl = slice(s * 512, (s + 1) * 512)
                            nc.tensor.matmul(kp[0:1, sl], kb_r, ones_r[0:1, sl],
                                             start=False, stop=(s == 1))
                        neginf = phd3.tile([1, N], F32, tag="neginf")
                        nc.vector.memset(neginf, float("-inf"))
                        res = phd3.tile([1, N], F32, tag="res")
                        nc.vector.select(out=res, mask=nm_sb, on_true=kp[0:1, :],
                                         on_false=neginf)
                        nc.sync.dma_start(out=out_d, in_=res)

    nc.compile()
    return nc


def _layer1_head(nc, tc, h, whT, adjT, al_sb, ones_r, ident_r, ident_bf, x2T):
    with tc.tile_pool(name=f"c{h}", bufs=1) as phc:
        f_sb = phc.tile([1, N], F32R, tag="f1")
        g_sb = phc.tile([1, N], F32R, tag="g1")
        wh = phc.tile([P, NJ, D], F32, tag="wh")
        with tc.tile_pool(name=f"c{h}_pre", bufs=2, space="PSUM") as pre_ps:
            # f, g rows: [1, N] = a^T @ Wh^T (K = 128)
            for col, dest in ((0, f_sb), (1, g_sb)):
                for s in range(2):
                    sl = slice(s * 512, (s + 1) * 512)
                    fp = pre_ps.tile([1, 512], F32, tag="fg")
                    nc.tensor.matmul(fp, al_sb[:, h, col:col + 1], whT[:, h, sl],
                                     start=True, stop=True)
                    nc.any.tensor_copy(dest[0:1, sl], fp)
            # Wh [j, d] = transpose(Wh^T)
            for jg in range(NJ // 4):
                tp = pre_ps.tile([P, 512], F32R, tag="whtr")
                for q in range(4):
                    jc = jg * 4 + q
                    nc.tensor.transpose(tp[:, q * P:(q + 1) * P],
                                        whT[:, h, jc * P:(jc + 1) * P], ident_r)
                nc.any.tensor_copy(
                    wh[:, jg * 4:(jg + 1) * 4, :].rearrange("p a b -> p (a b)"), tp)

        with tc.tile_pool(name=f"c{h}_w", bufs=1) as work, \
             tc.tile_pool(name=f"c{h}_ps", bufs=2, space="PSUM") as ep_ps, \
             tc.tile_pool(name=f"c{h}_hp", bufs=1, space="PSUM") as hp_ps:
            hp = hp_ps.tile([P, N], F32, tag="hp")
            for jc in range(NJ):
                mx = work.tile([P, N], F32R, tag="mx", bufs=2)
                rc = work.tile([P, 1], F32, tag="rc", bufs=2)
                _attention_tiles(nc, tc, ep_ps, work, f_sb, g_sb, adjT, ones_r,
                                 ident_bf, mx, rc, jc, "a")
                whp = work.tile([P, D], F32R, tag="whp", bufs=2)
                nc.vector.tensor_scalar(out=whp, in0=wh[:, jc, :], scalar1=rc,
                                        scalar2=None, op0=Alu.mult)
                for s in range(2):
                    sl = slice(s * 512, (s + 1) * 512)
                    nc.tensor.matmul(hp[:, sl], whp, mx[:, sl],
                                     start=(jc == 0), stop=(jc == NJ - 1))
            # elu(hp) -> x2T[:, h, :]
            es = work.tile([P, N], F32, tag="es")
            nc.scalar.activation(out=es, in_=hp, func=Act.Exp)
            tm = work.tile([P, N], F32, tag="tm")
            nc.vector.tensor_scalar(out=tm, in0=es, scalar1=-1.0, scalar2=0.0,
                                    op0=Alu.add, op1=Alu.min)
            nc.vector.scalar_tensor_tensor(out=x2T[:, h, :], in0=hp, scalar=0.0,
                                           in1=tm, op0=Alu.max, op1=Alu.add)


_NC_CACHE = None


def _get_nc():
    global _NC_CACHE
    if _NC_CACHE is None:
        _NC_CACHE = _build_nc()
    return _NC_CACHE


def kernel(graph_node, adj, node_mask, W, a1, a2, Wo, ao1, ao2, Wp, bp, kw, kb,
           **_unused):
    nc = _get_nc()
    graph_node = np.ascontiguousarray(np.asarray(graph_node, dtype=np.float32))
    adj = np.ascontiguousarray(np.asarray(adj, dtype=np.int32))
    node_mask = np.ascontiguousarray(np.asarray(node_mask, dtype=np.int32))
    shared = {
        "W": np.ascontiguousarray(np.asarray(W, np.float32)),
        "a1": np.ascontiguousarray(np.asarray(a1, np.float32)),
        "a2": np.ascontiguousarray(np.asarray(a2, np.float32)),
        "Wo": np.ascontiguousarray(np.asarray(Wo, np.float32)),
        "ao1": np.ascontiguousarray(np.asarray(ao1, np.float32).reshape(1, HID)),
        "ao2": np.ascontiguousarray(np.asarray(ao2, np.float32).reshape(1, HID)),
        "Wp": np.ascontiguousarray(np.asarray(Wp, np.float32)),
        "bp": np.ascontiguousarray(np.asarray(bp, np.float32).reshape(1, HID)),
        "kw": np.ascontiguousarray(np.asarray(kw, np.float32).reshape(HID, 1)),
        "kb": np.ascontiguousarray(np.asarray(kb, np.float32).reshape(1, 1)),
    }
    in_maps = []
    for b in range(BS):
        m = dict(shared)
        m["x"] = graph_node[b]
        m["adj"] = adj[b]
        m["nm"] = node_mask[b:b + 1]
        in_maps.append(m)

    res = run_bass_kernel_spmd(nc, in_maps, core_ids=list(range(BS)), trace=TRACE)
    out = np.concatenate([res.results[b]["out"] for b in range(BS)], axis=0)
    if TRACE:
        kernel.last_results = res
    return out
